# revision 2
# baseline (speedup 1.0000x reference)
"""Causal self-attention (RoPE, B=4 S=2048 D=2048 H=16) on 8 Trainium2 cores, v3.

Sharding: core c = 2*b + hh -> batch b, head-half hh (8 heads/core).
Host sums the two output-projection partials per batch.

Precision/speed scheme (measured rates: bf16 214ns, fp8-DR 74ns per
[128x512]-out matmul):
- Projections: 3-term split-fp8 DoubleRow: x8@Whi + x8@Wlo + xlo@Whi
  (lo terms live in fp8 denormal range; PSUM f32 accumulates all terms
  at one consistent scale).  ~Clean q,k,v at ~3x bf16 speed.
- QK^T: bf16 (contraction=128, DoubleRow not applicable).
- exp: Act engine, PSUM->bf16, scale folds 1/sqrt(hd) and the 64x
  host weight prescale; causal masking multiplies bf16 {0,1} masks on
  diagonal blocks only (off-diagonal key blocks skipped structurally).
- AV + denominator: bf16 matmuls on the bf16 exp output (fp8-splitting
  pt here moves the cost to DVE and loses more than it saves on PE).
- Output projection: 3-term split-fp8 DoubleRow over (head, hd) pairs.
"""
import sys

try:
    import concourse.bass as _chk  # noqa: F401
except ImportError:
    for p in ("/opt/trn_rl_repo", "/root/.axon_site/_ro/trn_rl_repo"):
        if p not in sys.path:
            sys.path.insert(0, p)

import math
import numpy as np
import ml_dtypes

import concourse.bass as bass
import concourse.tile as tile
from concourse import mybir
from concourse.bass_utils import run_bass_kernel_spmd

N_CORES = 8
B = 4
D = 2048
H = 16
HD = 128
HPC = 8
ROPE_BASE = 10000.0
F32 = mybir.dt.float32
BF16 = mybir.dt.bfloat16
FP8 = mybir.dt.float8e4
EXP = mybir.ActivationFunctionType.Exp
COPY = mybir.ActivationFunctionType.Copy
DR = mybir.MatmulPerfMode.DoubleRow
WS = 64.0
SCALE = 1.0 / math.sqrt(HD)
ESCALE = SCALE / (WS * WS)
NPF8 = ml_dtypes.float8_e4m3
NPBF = ml_dtypes.bfloat16


def split_ctrl_waits(nc, maxw=1):
    """Walrus in this env can't encode >1 sem-wait on many instruction
    formats; move extras onto preceding same-engine NoOps."""
    nid = [0]
    for f in nc.m.functions:
        for b in f.blocks:
            new_insts = []
            for inst in b.instructions:
                si = inst.sync_info
                if si is not None and si.on_wait is not None and len(si.on_wait) > maxw:
                    waits = list(si.on_wait)
                    while len(waits) > maxw:
                        chunk, waits = waits[:maxw], waits[maxw:]
                        nid[0] += 1
                        nop = mybir.InstNoOp(
                            name=f"I-waitsplit-{nid[0]}",
                            ins=[], outs=[],
                            sync_info=mybir.SyncInfo(on_wait=chunk, on_update=[]),
                        )
                        nop.engine = inst.engine
                        new_insts.append(nop)
                    si.on_wait = waits
                new_insts.append(inst)
            b.instructions[:] = new_insts


def build_nc(S=2048, repeat=1):
    KT = D // 128
    NQ = S // 512
    NK = S // 128

    nc = bass.Bass("TRN2", debug=False, num_devices=N_CORES)

    xhi_d = nc.dram_tensor("xhi", [KT, 128, S], FP8, kind="ExternalInput")
    xlo_d = nc.dram_tensor("xlo", [KT, 128, S], FP8, kind="ExternalInput")
    # per-head packed weights [h][i][k][o]
    w_d = {}
    for nm in ("wqh", "wql", "wkh", "wkl", "wvh", "wvl"):
        w_d[nm] = nc.dram_tensor(nm, [HPC, 128, KT, 128], FP8, kind="ExternalInput")
    woh_d = nc.dram_tensor("woh", [HPC, 128, D], FP8, kind="ExternalInput")
    wol_d = nc.dram_tensor("wol", [HPC, 128, D], FP8, kind="ExternalInput")
    cosf = nc.dram_tensor("cosf", [128, S], BF16, kind="ExternalInput")
    sinf = nc.dram_tensor("sinf", [128, S], BF16, kind="ExternalInput")
    dmask_d = nc.dram_tensor("dmask", [128, 4 * 512], BF16, kind="ExternalInput")
    ones_d = nc.dram_tensor("ones", [128, 128], BF16, kind="ExternalInput")
    ident_d = nc.dram_tensor("ident", [128, 128], BF16, kind="ExternalInput")
    out = nc.dram_tensor("out", [S, D], F32, kind="ExternalOutput")

    with tile.TileContext(nc) as tc:
        with tc.tile_pool(name="const", bufs=1) as cp:
            xhi = cp.tile([128, KT, S], FP8, name="xhi_t")
            xlo = cp.tile([128, KT, S], FP8, name="xlo_t")
            wosh = cp.tile([128, HPC, D], FP8, name="wosh")
            wosl = cp.tile([128, HPC, D], FP8, name="wosl")
            cos_t = cp.tile([128, S], BF16, name="cos_t")
            sin_t = cp.tile([128, S], BF16, name="sin_t")
            dmask = cp.tile([128, 4 * 512], BF16, name="dmask_t")
            ones2 = cp.tile([128, 128], BF16, name="ones2")
            ident = cp.tile([128, 128], BF16, name="ident_t")
            yhi = cp.tile([128, HPC, S], FP8, name="yhi")
            ylo = cp.tile([128, HPC, S], FP8, name="ylo")

            for _rep in range(repeat):
                for k in range(KT):
                    nc.sync.dma_start(xhi[:, k], xhi_d[k])
                    nc.sync.dma_start(xlo[:, k], xlo_d[k])
                nc.sync.dma_start(cos_t[:], cosf[:])
                nc.sync.dma_start(sin_t[:], sinf[:])
                nc.sync.dma_start(dmask[:], dmask_d[:])
                nc.sync.dma_start(ones2[:], ones_d[:])
                nc.sync.dma_start(ident[:], ident_d[:])
                for hh in range(HPC):
                    nc.sync.dma_start(wosh[:, hh], woh_d[hh])
                    nc.sync.dma_start(wosl[:, hh], wol_d[hh])
                _body(nc, tc, S, KT, NQ, NK, xhi, xlo, w_d, wosh, wosl,
                      cos_t, sin_t, dmask, ones2, ident, yhi, ylo, out)

    split_ctrl_waits(nc)
    return nc


def _body(nc, tc, S, KT, NQ, NK, xhi, xlo, w_d, wosh, wosl,
          cos_t, sin_t, dmask, ones2, ident, yhi, ylo, out):
    with tc.tile_pool(name="wst", bufs=2) as wst, \
         tc.tile_pool(name="hb", bufs=2) as hb, \
         tc.tile_pool(name="qkp", bufs=1) as qkp, \
         tc.tile_pool(name="vt1", bufs=1) as vt1, \
         tc.tile_pool(name="ptb", bufs=1) as ptb, \
         tc.tile_pool(name="pt8", bufs=1) as pt8, \
         tc.tile_pool(name="recp", bufs=1) as recp, \
         tc.tile_pool(name="pp", bufs=2, space="PSUM") as pp, \
         tc.tile_pool(name="ptr", bufs=2, space="PSUM") as ptr, \
         tc.tile_pool(name="pss", bufs=2, space="PSUM") as pss, \
         tc.tile_pool(name="psy", bufs=1, space="PSUM") as psy, \
         tc.tile_pool(name="psd", bufs=1, space="PSUM") as psd:

        for h in range(HPC):
            # ---- stream this head's weights ----
            w = {}
            for nm in ("wqh", "wql", "wkh", "wkl", "wvh", "wvl"):
                t = wst.tile([128, KT, 128], FP8, name=f"{nm}{h}", tag=nm)
                nc.sync.dma_start(t[:], w_d[nm][h])
                w[nm] = t

            # ---- projections: 3-term split-fp8 DR ----
            qraw = qkp.tile([128, S], BF16, name=f"qraw{h}", tag="qraw")
            kraw = qkp.tile([128, S], BF16, name=f"kraw{h}", tag="kraw")
            vT = vt1.tile([128, S], BF16, name=f"vT{h}", tag="vT")
            for dst, whi_, wlo_, sc in ((qraw, w["wqh"], w["wql"], None),
                                        (kraw, w["wkh"], w["wkl"], None),
                                        (vT, w["wvh"], w["wvl"], 1.0 / WS)):
                for cx in range(NQ):
                    po = pp.tile([128, 512], F32, name=f"po{h}_{cx}", tag="proj")
                    terms = ((whi_, xhi), (wlo_, xhi), (whi_, xlo))
                    for t_i, (wt, xt) in enumerate(terms):
                        for i in range(KT // 2):
                            nc.tensor.matmul(
                                po[:],
                                wt[:, 2 * i:2 * i + 2, :],
                                xt[:, 2 * i:2 * i + 2, cx * 512:(cx + 1) * 512],
                                start=(t_i == 0 and i == 0),
                                stop=(t_i == 2 and i == KT // 2 - 1),
                                perf_mode=DR)
                    if sc is None:
                        nc.scalar.copy(dst[:, cx * 512:(cx + 1) * 512], po[:])
                    else:
                        nc.scalar.activation(dst[:, cx * 512:(cx + 1) * 512],
                                             po[:], COPY, scale=sc)

            # ---- v: transpose to natural layout (bf16) ----
            v8 = hb.tile([128, NK, 128], BF16, name=f"v8_{h}", tag="v8")
            for t in range(NK):
                tp = ptr.tile([128, 128], BF16, name=f"tp{h}_{t}", tag="tr")
                nc.tensor.transpose(tp[:], vT[:, t * 128:(t + 1) * 128], ident[:])
                nc.scalar.copy(v8[:, t], tp[:])

            # ---- RoPE in place (bf16); swap copies on Act, scratch = vT buf ----
            sw = vt1.tile([128, S], BF16, name=f"sw{h}", tag="vT")
            for src_t in (qraw, kraw):
                nc.scalar.copy(sw[0:64, :], src_t[64:128, :])
                nc.scalar.copy(sw[64:128, :], src_t[0:64, :])
                nc.vector.tensor_mul(src_t[:], src_t[:], cos_t[:])
                nc.vector.tensor_mul(sw[:], sw[:], sin_t[:])
                nc.vector.tensor_add(src_t[:], src_t[:], sw[:])

            # ---- attention (bf16 QK / exp / AV / den) ----
            for qg in range(NQ):
                nkt = 4 * qg + 4
                pts = []
                for kt in range(nkt):
                    sps = pss.tile([128, 512], F32, name=f"sps{h}_{qg}_{kt}",
                                   tag="sps")
                    nc.tensor.matmul(sps[:], kraw[:, kt * 128:(kt + 1) * 128],
                                     qraw[:, qg * 512:(qg + 1) * 512],
                                     start=True, stop=True)
                    ptb_t = ptb.tile([128, 512], BF16, name=f"pt{h}_{qg}_{kt}",
                                     tag=f"pt{kt}")
                    nc.scalar.activation(ptb_t[:], sps[:], EXP, scale=ESCALE)
                    jj = kt - 4 * qg
                    if jj >= 0:
                        nc.vector.tensor_mul(ptb_t[:], ptb_t[:],
                                             dmask[:, jj * 512:(jj + 1) * 512])
                    pts.append(ptb_t)
                yps = psy.tile([128, 512], F32, name=f"yps{h}_{qg}", tag="yps")
                dps = psd.tile([128, 512], F32, name=f"dps{h}_{qg}", tag="dps")
                for kt in range(nkt):
                    nc.tensor.matmul(dps[:], ones2[:], pts[kt][:],
                                     start=(kt == 0), stop=(kt == nkt - 1))
                for kt in range(nkt):
                    nc.tensor.matmul(yps[:], v8[:, kt], pts[kt][:],
                                     start=(kt == 0), stop=(kt == nkt - 1))
                rec = recp.tile([128, 512], F32, name=f"rec{h}_{qg}", tag="rec")
                nc.vector.reciprocal(rec[:], dps[:])
                yb = ptb.tile([128, 512], BF16, name=f"yb{h}_{qg}", tag="yb")
                nc.vector.tensor_mul(yb[:], yps[:], rec[:])
                sl = slice(qg * 512, (qg + 1) * 512)
                nc.vector.tensor_copy(yhi[:, h, sl], yb[:])
                nc.vector.tensor_sub(ylo[:, h, sl], yb[:], yhi[:, h, sl])

    # ---- output projection: 3-term split-fp8 DR over (h, hd) ----
    with tc.tile_pool(name="cop", bufs=2) as cop, \
         tc.tile_pool(name="cps", bufs=2, space="PSUM") as cps:
        for st in range(NK):
            pos = [cps.tile([128, 512], F32, name=f"cpo{st}_{i}", tag=f"cpo{i}")
                   for i in range(4)]
            terms = ((yhi, wosh), (ylo, wosh), (yhi, wosl))
            for oc in range(4):
                c = 0
                nmm = 3 * (HPC // 2)
                for t_i, (yt, wt) in enumerate(terms):
                    for hp in range(HPC // 2):
                        nc.tensor.matmul(
                            pos[oc][:],
                            yt[:, 2 * hp:2 * hp + 2, st * 128:(st + 1) * 128],
                            wt[:, 2 * hp:2 * hp + 2, oc * 512:(oc + 1) * 512],
                            start=(c == 0), stop=(c == nmm - 1),
                            perf_mode=DR)
                        c += 1
            for oc in range(4):
                ot = cop.tile([128, 512], F32, name=f"cot{st}_{oc}", tag="cot")
                nc.scalar.activation(ot[:], pos[oc][:], COPY, scale=1.0 / WS)
                nc.sync.dma_start(out[st * 128:(st + 1) * 128,
                                      oc * 512:(oc + 1) * 512], ot[:])


def _split8(a):
    hi = a.astype(NPF8)
    lo = (a - hi.astype(np.float32)).astype(NPF8)
    return hi, lo


def prep_in_maps(x, positions, Wqkv, Wout, S=2048):
    KT = D // 128
    QF = HPC * HD

    inv_freq = 1.0 / (ROPE_BASE ** (np.arange(0, HD, 2, dtype=np.float64) / HD))
    pos = np.asarray(positions).astype(np.float64)[:S]
    freq = pos[None, :] * inv_freq[:, None]
    c = np.cos(freq).astype(np.float32)
    s = np.sin(freq).astype(np.float32)
    cosf = np.vstack([c, c]).astype(NPBF)
    sinf = np.vstack([-s, s]).astype(NPBF)

    dm = np.zeros((128, 4, 512), np.float32)
    for j in range(4):
        dm[:, j, 128 * j:128 * (j + 1)] = np.triu(np.ones((128, 128), np.float32))
        dm[:, j, 128 * (j + 1):] = 1.0
    dmask = dm.reshape(128, 4 * 512).astype(NPBF)

    ones8 = np.ones((128, 128), NPBF)
    ident = np.eye(128, dtype=np.float32).astype(NPBF)

    perm = np.concatenate([np.arange(0, HD, 2), np.arange(1, HD, 2)])

    in_maps = []
    for c_id in range(N_CORES):
        b, hh = c_id // 2, c_id % 2
        xT = np.ascontiguousarray(x[b, :S, :].T)       # [D, S] f32
        xhi = xT.astype(NPF8)
        xlo = (xT - xhi.astype(np.float32)).astype(NPF8)
        f0 = hh * QF
        Wq = Wqkv[:, f0:f0 + QF] * WS
        Wk = Wqkv[:, D + f0:D + f0 + QF] * WS
        Wv = Wqkv[:, 2 * D + f0:2 * D + f0 + QF] * WS
        Wqp = Wq.reshape(D, HPC, HD)[:, :, perm]       # [D, h, o]
        Wkp = Wk.reshape(D, HPC, HD)[:, :, perm]
        Wvp = Wv.reshape(D, HPC, HD)

        def pack(Wn):
            # [D, h, o] -> [h][i][k][o]
            return np.ascontiguousarray(
                Wn.reshape(KT, 128, HPC, HD).transpose(2, 1, 0, 3))

        wqh, wql = _split8(pack(Wqp))
        wkh, wkl = _split8(pack(Wkp))
        wvh, wvl = _split8(pack(Wvp))
        Woh = Wout[f0:f0 + QF, :] * WS                 # [1024, D]
        woh, wol = _split8(Woh.reshape(HPC, 128, D))
        in_maps.append({
            "xhi": xhi.reshape(KT, 128, S), "xlo": xlo.reshape(KT, 128, S),
            "wqh": wqh, "wql": wql, "wkh": wkh, "wkl": wkl,
            "wvh": wvh, "wvl": wvl, "woh": woh, "wol": wol,
            "cosf": cosf, "sinf": sinf, "dmask": dmask, "ones": ones8,
            "ident": ident,
        })
    return in_maps


def kernel(x, positions, mask, Wqkv, Wout):
    x = np.asarray(x, dtype=np.float32)
    Wqkv = np.asarray(Wqkv, dtype=np.float32)
    Wout = np.asarray(Wout, dtype=np.float32)
    S = x.shape[1]
    nc = build_nc(S=S)
    in_maps = prep_in_maps(x, positions, Wqkv, Wout, S=S)
    res = run_bass_kernel_spmd(nc, in_maps, core_ids=list(range(N_CORES)))
    outs = [res.results[c]["out"] for c in range(N_CORES)]
    full = np.stack([outs[2 * b] + outs[2 * b + 1] for b in range(B)], axis=0)
    return full.astype(np.float32)
